# revision 8
# baseline (speedup 1.0000x reference)
"""Multi-head attention (B=4, S=1024, D=1024, H=16) on 8 TRN2 NeuronCores.

Sharding: hybrid batch x head-group tensor parallel. Core c handles
batch b = c // 2 and head group g = c % 2 (8 heads, 512 feature dims).
Each core:
  - projects its batch's Q/K/V against its 512-row weight slices
    (feature-major layout: [feat, tok]),
  - computes scores [i, j] per head, exp via ScalarE with fused row-sum
    (accum_out), normalizes with per-partition reciprocal,
  - writes attn rows straight to DRAM in natural layout,
  - transposes attn tiles on TensorE to feed the attn @ V matmul,
  - computes its partial output projection (contraction over its 512 dims).
Host sums the two partials per batch and adds bo.
"""

import numpy as np

import concourse.bass as bass
import concourse.mybir as mybir
import concourse.tile as tile
from concourse.masks import make_identity

F32 = mybir.dt.float32

S = 1024          # sequence length
D = 1024          # d_model
HPC = 8           # heads per core
DK = 64           # head dim
F = 512           # feature dims per core (HPC * DK)
P = 128           # partitions
NCORES = 8


def _emit(tc, x_ps, w_ps, b_ps, wo_p, attn_o, out_o, ctx):
    nc = tc.nc

    const = ctx.enter_context(tc.tile_pool(name="const", bufs=1))
    ident = const.tile([P, P], F32, tag="ident")
    make_identity(nc, ident)
    zbias = const.tile([P, 1], F32, tag="zbias")
    nc.vector.memset(zbias, 0.0)

    # Per-projection bias tiles: [128, 4] with column fc = bias[fc*128:(fc+1)*128]
    bias_t = {}
    for nm, bp in b_ps.items():
        bt = const.tile([P, 4], F32, tag=f"bias_{nm}", name=f"bias_{nm}")
        for fc in range(4):
            nc.sync.dma_start(out=bt[:, fc : fc + 1], in_=bp[fc * P : (fc + 1) * P, :])
        bias_t[nm] = bt

    persist = ctx.enter_context(tc.tile_pool(name="persist", bufs=1))
    # weight slices, transposed to [k-part, feat] layout: [128, kc, 512]
    wT = {nm: persist.tile([P, 8, F], F32, tag=f"w{nm}T", name=f"w{nm}T")
          for nm in ("q", "k", "v")}
    # projected activations, feature-major: [128, fc, 1024]
    pT = {nm: persist.tile([P, 4, S], F32, tag=f"{nm}T", name=f"{nm}T")
          for nm in ("q", "k", "v")}
    # XT slot doubles as attnT (same 4MB footprint)
    ctxT = persist.tile([P, 4, S], F32, tag="ctxT")

    nat = ctx.enter_context(tc.tile_pool(name="nat", bufs=8))
    vtok_pool = ctx.enter_context(tc.tile_pool(name="vtok", bufs=2))
    small = ctx.enter_context(tc.tile_pool(name="small", bufs=8))

    psA = ctx.enter_context(tc.tile_pool(name="psA", bufs=2, space="PSUM"))
    psB = ctx.enter_context(tc.tile_pool(name="psB", bufs=2, space="PSUM"))

    # ---- Phase W: load + transpose weight slices --------------------------
    for nm in ("q", "k", "v"):
        wnat = []
        for fc in range(4):
            t = nat.tile([P, D], F32, tag="nat", name=f"w{nm}nat{fc}")
            nc.sync.dma_start(out=t, in_=w_ps[nm][fc * P : (fc + 1) * P, :])
            wnat.append(t)
        for kc in range(8):
            pt = psA.tile([P, 4 * P], F32, tag="psA", name=f"w{nm}tp{kc}")
            for fc in range(4):
                nc.tensor.transpose(
                    pt[:, fc * P : (fc + 1) * P],
                    wnat[fc][:, kc * P : (kc + 1) * P],
                    ident,
                )
            nc.vector.tensor_copy(out=wT[nm][:, kc, :], in_=pt)

    # ---- Phase P: per input, transpose X then project ---------------------
    for nm in ("q", "k", "v"):
        xnat = []
        for tc8 in range(8):
            t = nat.tile([P, D], F32, tag="nat", name=f"x{nm}nat{tc8}")
            nc.sync.dma_start(out=t, in_=x_ps[nm][tc8 * P : (tc8 + 1) * P, :])
            xnat.append(t)
        XT = persist.tile([P, 8, S], F32, tag="XT", name=f"X{nm}T")
        for kc in range(8):
            for th in range(2):
                pt = psA.tile([P, 4 * P], F32, tag="psA", name=f"x{nm}tp{kc}_{th}")
                for t in range(4):
                    nc.tensor.transpose(
                        pt[:, t * P : (t + 1) * P],
                        xnat[4 * th + t][:, kc * P : (kc + 1) * P],
                        ident,
                    )
                nc.vector.tensor_copy(
                    out=XT[:, kc, th * 512 : (th + 1) * 512], in_=pt
                )
        # projection: out[feat_chunk, tok] += wT[kc][:, fc].T @ XT[kc]
        for fc in range(4):
            for th in range(2):
                pp = psA.tile([P, 512], F32, tag="psA", name=f"p{nm}{fc}_{th}")
                for kc in range(8):
                    nc.tensor.matmul(
                        pp,
                        wT[nm][:, kc, fc * P : (fc + 1) * P],
                        XT[:, kc, th * 512 : (th + 1) * 512],
                        start=(kc == 0),
                        stop=(kc == 7),
                    )
                nc.vector.tensor_scalar_add(
                    out=pT[nm][:, fc, th * 512 : (th + 1) * 512],
                    in0=pp,
                    scalar1=bias_t[nm][:, fc : fc + 1],
                )

    qT, kT, vT = pT["q"], pT["k"], pT["v"]

    # ---- Phase A: attention per head --------------------------------------
    for h in range(HPC):
        fc, r0 = h // 2, (h % 2) * DK
        # v in token-major layout for this head: [128 j, jc, 64]
        v_tok = vtok_pool.tile([P, 8, DK], F32, tag="vtok", name=f"vtok{h}")
        for jc in range(8):
            pt = psA.tile([P, 4 * P], F32, tag="psA", name=f"vt{h}_{jc}")
            nc.tensor.transpose(
                pt[:, :DK],
                vT[r0 : r0 + DK, fc, jc * P : (jc + 1) * P],
                ident[r0 : r0 + DK, r0 : r0 + DK],
            )
            nc.vector.tensor_copy(out=v_tok[:, jc, :], in_=pt[:, :DK])

        attnT = persist.tile([P, 8, S], F32, tag="XT", name=f"attnT{h}")
        for ig in range(2):  # i-chunk groups of 4
            attn_ts = []
            for t4 in range(4):
                ic = ig * 4 + t4
                ps = psB.tile([P, S], F32, tag="psB", name=f"s{h}_{ic}")
                lhs = qT[r0 : r0 + DK, fc, ic * P : (ic + 1) * P]
                for jh in range(2):
                    nc.tensor.matmul(
                        ps[:, jh * 512 : (jh + 1) * 512],
                        lhs,
                        kT[r0 : r0 + DK, fc, jh * 512 : (jh + 1) * 512],
                        start=True,
                        stop=True,
                    )
                exp_s = nat.tile([P, S], F32, tag="nat", name=f"e{h}_{ic}")
                sums = small.tile([P, 1], F32, tag="small", name=f"sum{h}_{ic}")
                nc.scalar.activation(
                    out=exp_s,
                    in_=ps,
                    func=mybir.ActivationFunctionType.Exp,
                    bias=zbias,
                    scale=0.125,
                    accum_out=sums,
                )
                recip = small.tile([P, 1], F32, tag="small", name=f"rc{h}_{ic}")
                nc.vector.reciprocal(out=recip, in_=sums)
                attn_t = nat.tile([P, S], F32, tag="nat", name=f"a{h}_{ic}")
                nc.vector.tensor_scalar_mul(out=attn_t, in0=exp_s, scalar1=recip)
                nc.sync.dma_start(
                    out=attn_o[h, ic * P : (ic + 1) * P, :], in_=attn_t
                )
                attn_ts.append(attn_t)
            # transpose the 4 fresh i-chunks into attnT[:, jc, ig*512:+512]
            for jc in range(8):
                pt = psA.tile([P, 4 * P], F32, tag="psA", name=f"at{h}_{ig}_{jc}")
                for t4 in range(4):
                    nc.tensor.transpose(
                        pt[:, t4 * P : (t4 + 1) * P],
                        attn_ts[t4][:, jc * P : (jc + 1) * P],
                        ident,
                    )
                nc.vector.tensor_copy(
                    out=attnT[:, jc, ig * 512 : (ig + 1) * 512], in_=pt
                )

        # ctx^T for this head: [64 d, 1024 i] = sum_j v_tok[j, d] * attnT[j, i]
        pc = psB.tile([P, S], F32, tag="psB", name=f"ctx{h}")
        for jc in range(8):
            for ih in range(2):
                nc.tensor.matmul(
                    pc[:DK, ih * 512 : (ih + 1) * 512],
                    v_tok[:, jc, :],
                    attnT[:, jc, ih * 512 : (ih + 1) * 512],
                    start=(jc == 0),
                    stop=(jc == 7),
                )
        nc.vector.tensor_copy(out=ctxT[r0 : r0 + DK, fc, :], in_=pc[:DK, :])

    # ---- Phase O: output projection ---------------------------------------
    wonat = []
    for fc8 in range(8):
        t = nat.tile([P, F], F32, tag="wonat", name=f"wonat{fc8}")
        nc.sync.dma_start(out=t, in_=wo_p[fc8 * P : (fc8 + 1) * P, :])
        wonat.append(t)
    woT = persist.tile([P, 4, D], F32, tag="wqT", name="woT")
    for dc in range(4):
        pt = psB.tile([P, S], F32, tag="psB", name=f"wot{dc}")
        for fc8 in range(8):
            nc.tensor.transpose(
                pt[:, fc8 * P : (fc8 + 1) * P],
                wonat[fc8][:, dc * P : (dc + 1) * P],
                ident,
            )
        nc.vector.tensor_copy(out=woT[:, dc, :], in_=pt)

    for ic in range(8):
        po = psB.tile([P, S], F32, tag="psB", name=f"o{ic}")
        for dc in range(4):
            lhs = ctxT[:, dc, ic * P : (ic + 1) * P]
            for fh in range(2):
                nc.tensor.matmul(
                    po[:, fh * 512 : (fh + 1) * 512],
                    lhs,
                    woT[:, dc, fh * 512 : (fh + 1) * 512],
                    start=(dc == 0),
                    stop=(dc == 3),
                )
        out_sb = nat.tile([P, D], F32, tag="nat", name=f"osb{ic}")
        nc.vector.tensor_copy(out=out_sb, in_=po)
        nc.sync.dma_start(out=out_o[ic * P : (ic + 1) * P, :], in_=out_sb)


_SYNC_SPLIT_N = [0]


def _legalize_sync(nc):
    """Split multi-wait sync_info into standalone EventSemaphore instructions.

    The walrus build in this container rejects instructions carrying more
    than one wait (+ one update) in their 64-byte encoding ("Too many sync
    wait commands").  A standalone wait on the same engine immediately
    before the instruction is semantically identical.
    """
    for fn in nc.m.functions:
        for bb in fn.blocks:
            insts = list(bb.instructions)
            out = []
            changed = False
            for inst in insts:
                si = inst.sync_info
                if si is not None and len(si.on_wait) > 1 and \
                        inst.engine != mybir.EngineType.Unassigned:
                    waits = list(si.on_wait)
                    for w in waits[:-1]:
                        _SYNC_SPLIT_N[0] += 1
                        ev = mybir.InstEventSemaphore(
                            name=f"I-syncsplit-{_SYNC_SPLIT_N[0]}",
                            engine=inst.engine,
                            sync_info=mybir.SyncInfo(on_wait=[w], on_update=[]),
                        )
                        out.append(ev)
                    inst.sync_info = mybir.SyncInfo(
                        on_wait=[waits[-1]], on_update=list(si.on_update)
                    )
                    changed = True
                out.append(inst)
            if changed:
                bb.instructions = out


def _build_nc():
    nc = bass.Bass()
    x_ps = {nm: nc.declare_dram_parameter(f"x_{nm}", [S, D], F32, isOutput=False)
            for nm in ("q", "k", "v")}
    w_ps = {nm: nc.declare_dram_parameter(f"w{nm}", [F, D], F32, isOutput=False)
            for nm in ("q", "k", "v")}
    b_ps = {nm: nc.declare_dram_parameter(f"b{nm}", [F, 1], F32, isOutput=False)
            for nm in ("q", "k", "v")}
    wo_p = nc.declare_dram_parameter("wo", [D, F], F32, isOutput=False)
    attn_o = nc.declare_dram_parameter("attn_out", [HPC, S, S], F32, isOutput=True)
    out_o = nc.declare_dram_parameter("out_partial", [S, D], F32, isOutput=True)

    from contextlib import ExitStack
    with tile.TileContext(nc) as tc, ExitStack() as ctx:
        _emit(tc, x_ps, w_ps, b_ps, wo_p, attn_o, out_o, ctx)
    _legalize_sync(nc)
    return nc


_NC_CACHE = None


def _get_nc():
    global _NC_CACHE
    if _NC_CACHE is None:
        _NC_CACHE = _build_nc()
    return _NC_CACHE


def _make_in_maps(Q, K, V, wq, bq, wk, bk, wv, bv, wo):
    in_maps = []
    for c in range(NCORES):
        b, g = c // 2, c % 2
        fs = slice(g * F, (g + 1) * F)
        in_maps.append({
            "x_q": np.ascontiguousarray(Q[b], dtype=np.float32),
            "x_k": np.ascontiguousarray(K[b], dtype=np.float32),
            "x_v": np.ascontiguousarray(V[b], dtype=np.float32),
            "wq": np.ascontiguousarray(wq[fs], dtype=np.float32),
            "wk": np.ascontiguousarray(wk[fs], dtype=np.float32),
            "wv": np.ascontiguousarray(wv[fs], dtype=np.float32),
            "bq": np.ascontiguousarray(bq[fs], dtype=np.float32).reshape(F, 1),
            "bk": np.ascontiguousarray(bk[fs], dtype=np.float32).reshape(F, 1),
            "bv": np.ascontiguousarray(bv[fs], dtype=np.float32).reshape(F, 1),
            "wo": np.ascontiguousarray(wo[:, fs], dtype=np.float32),
        })
    return in_maps


def run(Q, K, V, wq, bq, wk, bk, wv, bv, wo, bo, trace=False, **spmd_kwargs):
    from concourse.bass_utils import run_bass_kernel_spmd

    nc = _get_nc()
    in_maps = _make_in_maps(Q, K, V, wq, bq, wk, bk, wv, bv, wo)
    res = run_bass_kernel_spmd(nc, in_maps, list(range(NCORES)), trace=trace,
                               **spmd_kwargs)

    B, H = 4, 16
    out = np.zeros((B, S, D), np.float32)
    attn = np.empty((B, H, S, S), np.float32)
    for c in range(NCORES):
        b, g = c // 2, c % 2
        out[b] += res.results[c]["out_partial"]
        attn[b, g * HPC : (g + 1) * HPC] = res.results[c]["attn_out"]
    out += np.asarray(bo, np.float32)
    return (out, attn), res


def kernel(Q, K, V, wq, bq, wk, bk, wv, bv, wo, bo):
    (out, attn), _ = run(Q, K, V, wq, bq, wk, bk, wv, bv, wo, bo)
    return out, attn
